# revision 20
# baseline (speedup 1.0000x reference)
"""Engram block (hash-embedding gather + gated value + dilated causal depthwise
conv) as a Bass/Tile SPMD kernel on 8 Trainium2 NeuronCores.

Sharding: sequence (L) split 8 ways; each core recomputes a 12-position halo
for the causal conv. Embedding tables are replicated (the gather reads only
needed rows). Weights host-transposed/cast to bf16.

v2 structure (two phases per core):
  Phase 1, per 128-token m-tile (9 tiles):
    - ONE batched indirect-DMA gathers all 12 head embeddings for the tile
      (SWDGE fixed cost is ~1us/instruction, so batching 12->1 matters)
    - 12 PE transposes -> emb_sb [e, m] (batched PSUM eviction, 4 at a time)
    - k-projection matmul with stationary=emb block, moving=Wk cols; PSUM is
      [m, d] so RMS/gate stats are free-dim reductions (ACT square-accum,
      DVE tensor_tensor accum against hidden rows)
  Gate tail: batched over all 9 tiles on [128, 9] stat tiles; gates are
  PE-transposed + broadcast (ones-block matmul) to a [*, m] bf16 row tile.
  Phase 2, per 128-row d-tile (16 tiles):
    - v-projection in [d, m] orientation: stationary=Wv block [e,d], moving
      = emb_sb [e, m] -> PSUM [d, m]; NO value transposes needed
    - gate applied during PSUM eviction (DVE tensor_tensor vs broadcast gates)
    - dilated causal conv = 4 full-width (1024) DVE shifted fused ops, bf16
    - output written as [D, m] bf16; host re-transposes/upcasts
"""
import sys

sys.path.insert(0, "/opt/trn_rl_repo")

import numpy as np
import ml_dtypes

import concourse.bass as bass
import concourse.tile as tile
from concourse import mybir
from concourse.masks import make_identity
from concourse.bass_utils import run_bass_kernel_spmd

# problem shapes (hardcoded per spec)
L, B, D = 4096, 2, 2048
H, Dh = 12, 128
E = H * Dh  # 1536
N = 100000
K, DIL = 4, 4
EPS = 1e-6

NCORES = 8
LC = L // NCORES          # 512 l-positions per core
HALO = (K - 1) * DIL      # 12
LE = LC + HALO            # 524
M = LE * B                # 1048 valid tokens (l-major, b inner)
MP = 1152                 # padded to 9*128
MT = MP // 128            # 9 m-tiles
DT = D // 128             # 16 d-tiles
ET = E // 128             # 12 e-tiles
MOUT = LC * B             # 1024 output tokens per core
OFF = HALO * B            # 24 = first valid output token
# first NEARLY m-tiles are computed [m,d]-oriented during phase-1 gather
# stalls; phase-2 [d,m] chunks cover the rest (PSUM bank is 512 fp32; the
# last chunk stops at token 1048 — columns beyond are pad no consumer reads)
NEARLY = 2
CHUNKS = [(256, 512), (768, 256), (1024, 24)]

BF16 = mybir.dt.bfloat16
F32 = mybir.dt.float32
I32 = mybir.dt.int32

# scal columns per d-tile
SC_W0, SC_W1, SC_W2, SC_W3P, SC_CB = range(5)
NSC = 5


def _split_multi_waits(nc):
    """This walrus build accepts only one sync-wait per instruction; hoist
    extra waits onto injected NOPs on the same engine (order-preserving)."""
    for f in nc.m.functions:
        for bb in f.blocks:
            new_insts = []
            for inst in bb.instructions:
                si = inst.sync_info
                if si is not None and si.on_wait and len(si.on_wait) > 1:
                    for w in si.on_wait[:-1]:
                        nop = mybir.InstNoOp(
                            name=nc.get_next_instruction_name(), ins=[], outs=[]
                        )
                        nop.engine = inst.engine
                        nop.sync_info = mybir.SyncInfo(on_wait=[w], on_update=[])
                        new_insts.append(nop)
                    si.on_wait = [si.on_wait[-1]]
                new_insts.append(inst)
            bb.instructions = new_insts


def build_program(split_waits=True, debug_dumps=False):
    nc = bass.Bass("TRN2", target_bir_lowering=False, debug=False)
    demb = dgb = None
    if debug_dumps:
        demb = nc.declare_dram_parameter(
            "demb", [128, ET * MP], BF16, isOutput=True
        )
        dgb = nc.declare_dram_parameter("dgb", [128, MP], BF16, isOutput=True)

    tabs = nc.declare_dram_parameter(
        "tabs", [H * N + 16, Dh], BF16, isOutput=False
    )
    ids = nc.declare_dram_parameter("ids", [128, MT * H], I32, isOutput=False)
    hid = nc.declare_dram_parameter("hid", [MP, D], BF16, isOutput=False)
    wk = nc.declare_dram_parameter("wk", [E, D], BF16, isOutput=False)
    wv = nc.declare_dram_parameter("wv", [E, D], BF16, isOutput=False)
    scal = nc.declare_dram_parameter("scal", [128, DT * NSC], F32, isOutput=False)
    e9 = nc.declare_dram_parameter("e9", [128, MP], BF16, isOutput=False)
    outT = nc.declare_dram_parameter("outT", [D, MOUT], BF16, isOutput=True)

    AR = mybir.ActivationFunctionType
    ALU = mybir.AluOpType

    with tile.TileContext(nc) as tc:
        with (
            tc.tile_pool(name="persist", bufs=1) as pp,
            tc.tile_pool(name="work", bufs=3) as wp,
            tc.tile_pool(name="stat", bufs=2) as sp,
        ):
            # ---- constants / small inputs ----
            eps_sb = pp.tile([128, 1], F32, tag="eps")
            nc.vector.memset(eps_sb[:], EPS)

            ids_sb = pp.tile([128, MT * H], I32, tag="ids")
            nc.sync.dma_start(ids_sb[:], ids.ap())
            scal_sb = pp.tile([128, DT * NSC], F32, tag="scal")
            nc.sync.dma_start(scal_sb[:], scal.ap())
            e9_sb = pp.tile([128, MP], BF16, tag="e9")
            nc.sync.dma_start(e9_sb[:], e9.ap())

            def sc(dt_, c):
                return scal_sb[:, dt_ * NSC + c : dt_ * NSC + c + 1]

            # ---- weights: wk first (phase 1 needs it), wv during phase 1 ----
            wk_sb = pp.tile([128, ET, D], BF16, tag="wk")
            for e in range(ET):
                nc.scalar.dma_start(wk_sb[:, e, :], wk[e * 128 : (e + 1) * 128, :])
            wv_sb = pp.tile([128, ET, D], BF16, tag="wv")
            for e in range(ET):
                nc.scalar.dma_start(wv_sb[:, e, :], wv[e * 128 : (e + 1) * 128, :])

            # ---- gathers: ONE batched indirect DMA per m-tile ----
            bc_reg = nc.gpsimd.to_reg(H * N - 1)
            emb_raws = []
            for t in range(MT):
                er = wp.tile(
                    [128, H, Dh], BF16, tag="emb_raw", bufs=4,
                    name=f"emb_raw{t}",
                )
                if t in (0, MT - 1):
                    nc.gpsimd.memset(er[:, :, :], 0)
                # one instruction per (tile, head): multi-column offset APs
                # mis-bind on this walrus build (all collapse onto the last
                # instruction's ids), so stay on the single-column path
                for h in range(H):
                    nc.gpsimd.indirect_dma_start(
                        out=er[:, h, :],
                        out_offset=None,
                        in_=tabs[:],
                        in_offset=bass.IndirectOffsetOnAxis(
                            ap=ids_sb[:, t * H + h : t * H + h + 1], axis=0
                        ),
                        bounds_check=bc_reg,
                        oob_is_err=False,
                    )
                emb_raws.append(er)

            ident = pp.tile([128, 128], BF16, tag="ident")
            make_identity(nc, ident[:])
            emb_sb = pp.tile([128, ET, MP], BF16, tag="emb")
            vg = pp.tile([128, DT, MP], BF16, tag="vg")
            g_t = pp.tile([128, MT], BF16, tag="g_t")
            g_tf = pp.tile([128, MT], F32, tag="g_tf")
            # per-tile stat columns (f32), consumed by the batched gate tail
            sk_a = pp.tile([128, MT], F32, tag="sk_a")
            sk_b = pp.tile([128, MT], F32, tag="sk_b")
            sh_t = pp.tile([128, MT], F32, tag="sh_t")
            pk_a = pp.tile([128, MT], F32, tag="pk_a")
            pk_b = pp.tile([128, MT], F32, tag="pk_b")


            def gate_tail(lo, hi):
                """signed-sqrt sigmoid gate for m-tiles [lo, hi) on stat cols."""
                w = hi - lo
                sk = sp.tile([128, w], F32, tag="sk", name=f"sk{lo}")
                nc.vector.tensor_add(sk[:], sk_a[:, lo:hi], sk_b[:, lo:hi])
                s1 = sp.tile([128, w], F32, tag="s1", name=f"s1_{lo}")
                nc.scalar.activation(
                    out=s1[:], in_=sk[:], func=AR.Identity,
                    bias=eps_sb[:, 0:1], scale=1.0 / D,
                )
                s2 = sp.tile([128, w], F32, tag="s2", name=f"s2_{lo}")
                nc.scalar.activation(
                    out=s2[:], in_=sh_t[:, lo:hi], func=AR.Identity,
                    bias=eps_sb[:, 0:1], scale=1.0 / D,
                )
                tt = sp.tile([128, w], F32, tag="tt", name=f"tt{lo}")
                nc.vector.tensor_mul(tt[:], s1[:], s2[:])
                rr = sp.tile([128, w], F32, tag="rr", name=f"rr{lo}")
                nc.vector.reciprocal(rr[:], tt[:])
                rq = sp.tile([128, w], F32, tag="rq", name=f"rq{lo}")
                nc.scalar.activation(out=rq[:], in_=rr[:], func=AR.Sqrt)
                pks = sp.tile([128, w], F32, tag="pks", name=f"pks{lo}")
                nc.vector.tensor_add(pks[:], pk_a[:, lo:hi], pk_b[:, lo:hi])
                uu = sp.tile([128, w], F32, tag="uu", name=f"uu{lo}")
                nc.vector.scalar_tensor_tensor(
                    out=uu[:], in0=pks[:], scalar=float(1.0 / np.sqrt(D)),
                    in1=rq[:], op0=ALU.mult, op1=ALU.mult,
                )
                ab = sp.tile([128, w], F32, tag="ab", name=f"ab{lo}")
                nc.scalar.activation(out=ab[:], in_=uu[:], func=AR.Abs)
                mx = sp.tile([128, w], F32, tag="mx", name=f"mx{lo}")
                nc.vector.tensor_scalar_max(out=mx[:], in0=ab[:], scalar1=1e-6)
                r2 = sp.tile([128, w], F32, tag="r2", name=f"r2_{lo}")
                nc.vector.reciprocal(r2[:], mx[:])
                q2 = sp.tile([128, w], F32, tag="q2", name=f"q2_{lo}")
                nc.scalar.activation(out=q2[:], in_=r2[:], func=AR.Sqrt)
                st = sp.tile([128, w], F32, tag="st", name=f"st{lo}")
                nc.vector.tensor_mul(st[:], uu[:], q2[:])
                nc.scalar.activation(
                    out=g_tf[:, lo:hi], in_=st[:], func=AR.Sigmoid
                )
                nc.scalar.copy(out=g_t[:, lo:hi], in_=g_tf[:, lo:hi])

            # ================= PHASE 1 =================
            with tc.tile_pool(name="psum1", bufs=1, space="PSUM") as ps1:
                for t in range(MT):
                    er = emb_raws[t]
                    # transpose 12 head blocks -> emb_sb, 4 per PSUM eviction
                    for grp in range(3):
                        pt = ps1.tile(
                            [128, 512], BF16, tag="tpose", bufs=2, space="PSUM"
                        )
                        for j in range(4):
                            h = grp * 4 + j
                            nc.tensor.transpose(
                                out=pt[:, j * 128 : (j + 1) * 128],
                                in_=er[:, h, :],
                                identity=ident[:],
                            )
                        nc.scalar.copy(
                            out=emb_sb[
                                :, grp * 4 : (grp + 1) * 4, t * 128 : (t + 1) * 128
                            ],
                            in_=pt[:],
                        )

                    # hidden rows for this m-tile + h^2 accum
                    h_md = wp.tile([128, D], BF16, tag="h_md", bufs=2)
                    nc.sync.dma_start(h_md[:], hid.ap()[t * 128 : (t + 1) * 128, :])
                    hsj = wp.tile([128, D], BF16, tag="junk", bufs=2, name="hsj")
                    nc.scalar.activation(
                        out=hsj[:], in_=h_md[:], func=AR.Square,
                        accum_out=sh_t[:, t : t + 1],
                    )

                    # k matmul in 2 col-groups of 1024 (2 PSUM banks each)
                    for g in range(2):
                        kp = ps1.tile(
                            [128, 1024], F32, tag="kp", bufs=3, space="PSUM"
                        )
                        for e in range(ET):
                            for b in range(2):
                                nc.tensor.matmul(
                                    out=kp[:, b * 512 : (b + 1) * 512],
                                    lhsT=emb_sb[:, e, t * 128 : (t + 1) * 128],
                                    rhs=wk_sb[
                                        :, e,
                                        g * 1024 + b * 512 : g * 1024 + (b + 1) * 512,
                                    ],
                                    start=(e == 0), stop=(e == ET - 1),
                                )
                        # k stats: sum k^2 (ACT), sum k*h (DVE)
                        ksj = wp.tile(
                            [128, 1024], BF16, tag="junk", bufs=2, name="ksj"
                        )
                        sk_dst = sk_a if g == 0 else sk_b
                        nc.scalar.activation(
                            out=ksj[:], in_=kp[:], func=AR.Square,
                            accum_out=sk_dst[:, t : t + 1],
                        )
                        khj = wp.tile(
                            [128, 1024], BF16, tag="junk", bufs=2, name="khj"
                        )
                        pk_dst = pk_a if g == 0 else pk_b
                        nc.vector.scalar_tensor_tensor(
                            out=khj[:], in0=kp[:], scalar=1.0,
                            in1=h_md[:, g * 1024 : (g + 1) * 1024],
                            op0=ALU.mult, op1=ALU.mult,
                            accum_out=pk_dst[:, t : t + 1],
                        )

                    # weave early [m,d]-oriented value matmuls for the first
                    # NEARLY tiles into the gather-paced stalls; gate applies
                    # as a per-partition scalar, PE transposes land in vg
                    if g == 1 and t == 2:
                        gate_tail(0, NEARLY)
                    if g == 1 and 4 <= t <= 7:
                        tau, pass_ = (t - 4) // 2, (t - 4) % 2
                        vmd = ps1.tile(
                            [128, 1024], F32, tag="kp", bufs=3, space="PSUM",
                            name=f"evmd{tau}_{pass_}",
                        )
                        for e in range(ET):
                            for b in range(2):
                                nc.tensor.matmul(
                                    out=vmd[:, b * 512 : (b + 1) * 512],
                                    lhsT=emb_sb[:, e, tau * 128 : (tau + 1) * 128],
                                    rhs=wv_sb[
                                        :, e,
                                        pass_ * 1024 + b * 512 :
                                        pass_ * 1024 + (b + 1) * 512,
                                    ],
                                    start=(e == 0), stop=(e == ET - 1),
                                )
                        evsb = wp.tile([128, 1024], BF16, tag="evsb", bufs=2)
                        nc.vector.tensor_scalar_mul(
                            out=evsb[:], in0=vmd[:],
                            scalar1=g_tf[:, tau : tau + 1],
                        )
                        for grp in range(2):
                            pt = ps1.tile(
                                [128, 512], BF16, tag="tpose", bufs=2,
                                space="PSUM", name=f"evtp{tau}_{pass_}_{grp}",
                            )
                            for j in range(4):
                                blk = grp * 4 + j
                                nc.tensor.transpose(
                                    out=pt[:, j * 128 : (j + 1) * 128],
                                    in_=evsb[:, blk * 128 : (blk + 1) * 128],
                                    identity=ident[:],
                                )
                            dt0 = pass_ * 8 + grp * 4
                            nc.scalar.copy(
                                out=vg[
                                    :, dt0 : dt0 + 4, tau * 128 : (tau + 1) * 128
                                ],
                                in_=pt[:],
                            )

            gate_tail(NEARLY, MT)

            # ================= PHASE 2 =================
            # the gate broadcast's PE ops (transpose + ones-block matmuls)
            # wait on the DVE/ACT gate tail; emit the first two d-tiles'
            # matmuls ahead of them so the PE queue never stalls there
            gb = pp.tile([128, MP], BF16, tag="gb")
            GB_CHUNKS = [(256, 512), (768, 384)]
            with tc.tile_pool(name="psum2", bufs=1, space="PSUM") as ps2:

                def v_mms(dt_):
                    vps = []
                    for ci, (c0, cw) in enumerate(CHUNKS):
                        vp = ps2.tile(
                            [128, cw], F32, tag=f"vp{ci}", bufs=2, space="PSUM",
                            name=f"vp{ci}_{dt_}",
                        )
                        vps.append(vp)
                    for e in range(ET):
                        for ci, (c0, cw) in enumerate(CHUNKS):
                            nc.tensor.matmul(
                                out=vps[ci][:, 0:cw],
                                lhsT=wv_sb[:, e, dt_ * 128 : (dt_ + 1) * 128],
                                rhs=emb_sb[:, e, c0 : c0 + cw],
                                start=(e == 0), stop=(e == ET - 1),
                            )
                    return vps

                def v_tail(dt_, vps):
                    # gate-apply during PSUM eviction
                    for ci, (c0, cw) in enumerate(CHUNKS):
                        nc.vector.tensor_mul(
                            vg[:, dt_, c0 : c0 + cw], vps[ci][:, 0:cw],
                            gb[:, c0 : c0 + cw],
                        )
                    # dilated causal depthwise conv: 4 full-width fused taps
                    a1 = wp.tile([128, MOUT], BF16, tag="a1", bufs=1)
                    nc.vector.tensor_scalar(
                        out=a1[:], in0=vg[:, dt_, 0:MOUT],
                        scalar1=sc(dt_, SC_W0), scalar2=sc(dt_, SC_CB),
                        op0=ALU.mult, op1=ALU.add,
                    )
                    a2 = wp.tile([128, MOUT], BF16, tag="a2", bufs=1)
                    nc.vector.scalar_tensor_tensor(
                        out=a2[:], in0=vg[:, dt_, 8 : 8 + MOUT],
                        scalar=sc(dt_, SC_W1), in1=a1[:],
                        op0=ALU.mult, op1=ALU.add,
                    )
                    a3 = wp.tile([128, MOUT], BF16, tag="a3", bufs=1)
                    nc.vector.scalar_tensor_tensor(
                        out=a3[:], in0=vg[:, dt_, 16 : 16 + MOUT],
                        scalar=sc(dt_, SC_W2), in1=a2[:],
                        op0=ALU.mult, op1=ALU.add,
                    )
                    ot = wp.tile([128, MOUT], BF16, tag="ot", bufs=1)
                    nc.vector.scalar_tensor_tensor(
                        out=ot[:], in0=vg[:, dt_, OFF : OFF + MOUT],
                        scalar=sc(dt_, SC_W3P), in1=a3[:],
                        op0=ALU.mult, op1=ALU.add,
                    )
                    nc.sync.dma_start(
                        outT[dt_ * 128 : (dt_ + 1) * 128, :], ot[:]
                    )

                held = [(dt_, v_mms(dt_)) for dt_ in range(2)]

                # gate broadcast: transpose gates, then ones-block matmuls
                gt_ps = ps2.tile([128, 128], BF16, tag="gt_ps", space="PSUM")
                nc.tensor.transpose(
                    out=gt_ps[0:MT, :], in_=g_t[:, 0:MT], identity=ident[:]
                )
                g_row = sp.tile([128, 128], BF16, tag="g_row")
                nc.scalar.copy(out=g_row[0:MT, :], in_=gt_ps[0:MT, :])
                for ci, (c0, cw) in enumerate(GB_CHUNKS):
                    gb_ps = ps2.tile(
                        [128, 512], F32, tag="gb_ps", bufs=1, space="PSUM"
                    )
                    for j in range(cw // 128):
                        t = c0 // 128 + j
                        nc.tensor.matmul(
                            out=gb_ps[:, j * 128 : (j + 1) * 128],
                            lhsT=e9_sb[0:MT, t * 128 : (t + 1) * 128],
                            rhs=g_row[0:MT, :],
                            start=True, stop=True,
                        )
                    nc.scalar.copy(out=gb[:, c0 : c0 + cw], in_=gb_ps[:, 0:cw])

                if debug_dumps:
                    nc.sync.dma_start(demb.ap(), emb_sb[:, :, :])
                    nc.sync.dma_start(dgb.ap(), gb[:])

                for dt_, vps in held:
                    v_tail(dt_, vps)
                for dt_ in range(2, DT):
                    v_tail(dt_, v_mms(dt_))

    if split_waits:
        _split_multi_waits(nc)
    return nc


_CACHE = {}


def _get_program():
    if "nc" not in _CACHE:
        _CACHE["nc"] = build_program()
    return _CACHE["nc"]


def host_prep(hidden_states, hash_input_ids, emb_tables, key_w, key_b,
              norm1_w, norm2_w, value_w, value_b, conv_w, conv_b):
    """Shard + lay out inputs for the 8 cores. Returns in_maps list."""
    bf = ml_dtypes.bfloat16
    w12 = norm1_w.astype(np.float64) * norm2_w.astype(np.float64)
    assert np.allclose(w12, 1.0, atol=1e-5), (
        "fast path assumes norm1_w*norm2_w == 1 (problem spec: fill=ones)"
    )
    assert not key_b.any() and not value_b.any(), (
        "fast path assumes zero key/value biases (problem spec: fill=zeros)"
    )

    tabs_np = np.zeros((H * N + 16, Dh), bf)
    tabs_np[: H * N] = emb_tables.reshape(H * N, Dh).astype(bf)
    wk_np = np.ascontiguousarray(key_w.T).astype(bf)
    wv_np = np.ascontiguousarray(value_w.T).astype(bf)
    scal_d = np.empty((D, NSC), np.float32)
    scal_d[:, SC_W0] = conv_w[:, 0]
    scal_d[:, SC_W1] = conv_w[:, 1]
    scal_d[:, SC_W2] = conv_w[:, 2]
    scal_d[:, SC_W3P] = conv_w[:, 3] + 1.0
    scal_d[:, SC_CB] = conv_b
    scal_np = np.ascontiguousarray(
        scal_d.reshape(DT, 128, NSC).transpose(1, 0, 2).reshape(128, DT * NSC)
    )
    e9_np = np.zeros((128, MP), bf)
    for t in range(MT):
        e9_np[t, t * 128 : (t + 1) * 128] = 1.0

    head_off = (np.arange(H, dtype=np.int64) * N)[None, :]
    OOB = np.int32(H * N)

    in_maps = []
    for c in range(NCORES):
        l0 = c * LC
        lo = l0 - HALO
        lo_clip = max(lo, 0)
        nvalid = (l0 + LC) - lo_clip
        r0 = (lo_clip - lo) * B
        ids_c = np.full((MP, H), OOB, np.int32)
        seg = hash_input_ids[lo_clip : l0 + LC].reshape(nvalid * B, H)
        ids_c[r0 : r0 + nvalid * B] = (seg.astype(np.int64) + head_off).astype(
            np.int32
        )
        hid_c = np.zeros((MP, D), bf)
        hseg = hidden_states[lo_clip : l0 + LC].reshape(nvalid * B, D)
        hid_c[r0 : r0 + nvalid * B] = hseg.astype(bf)
        ids_r = np.ascontiguousarray(
            ids_c.reshape(MT, 128, H).transpose(1, 0, 2).reshape(128, MT * H)
        )
        in_maps.append(
            {
                "tabs": tabs_np,
                "ids": ids_r,
                "hid": hid_c,
                "wk": wk_np,
                "wv": wv_np,
                "scal": scal_np,
                "e9": e9_np,
            }
        )
    return in_maps


def unshard_output(results):
    """results: list of per-core dicts with 'outT' [D, MOUT] -> [L, B, D]."""
    out = np.empty((L, B, D), np.float32)
    for c in range(NCORES):
        o = results[c]["outT"].astype(np.float32)
        out[c * LC : (c + 1) * LC] = o.reshape(D, LC, B).transpose(1, 2, 0)
    return out


def kernel(hidden_states, hash_input_ids, emb_tables, key_w, key_b,
           norm1_w, norm2_w, value_w, value_b, conv_w, conv_b):
    args = [hidden_states, hash_input_ids, emb_tables, key_w, key_b,
            norm1_w, norm2_w, value_w, value_b, conv_w, conv_b]
    args = [np.asarray(a) for a in args]
    in_maps = host_prep(*args)
    nc = _get_program()
    res = run_bass_kernel_spmd(nc, in_maps, list(range(NCORES)))
    return unshard_output(res.results)


# revision 21
# speedup vs baseline: 1.0108x; 1.0108x over previous
"""Engram block (hash-embedding gather + gated value + dilated causal depthwise
conv) as a Bass/Tile SPMD kernel on 8 Trainium2 NeuronCores.

Sharding: sequence (L) split 8 ways; each core recomputes a 12-position halo
for the causal conv. Embedding tables are replicated (the gather reads only
needed rows). Weights host-transposed/cast to bf16.

v2 structure (two phases per core):
  Phase 1, per 128-token m-tile (9 tiles):
    - ONE batched indirect-DMA gathers all 12 head embeddings for the tile
      (SWDGE fixed cost is ~1us/instruction, so batching 12->1 matters)
    - 12 PE transposes -> emb_sb [e, m] (batched PSUM eviction, 4 at a time)
    - k-projection matmul with stationary=emb block, moving=Wk cols; PSUM is
      [m, d] so RMS/gate stats are free-dim reductions (ACT square-accum,
      DVE tensor_tensor accum against hidden rows)
  Gate tail: batched over all 9 tiles on [128, 9] stat tiles; gates are
  PE-transposed + broadcast (ones-block matmul) to a [*, m] bf16 row tile.
  Phase 2, per 128-row d-tile (16 tiles):
    - v-projection in [d, m] orientation: stationary=Wv block [e,d], moving
      = emb_sb [e, m] -> PSUM [d, m]; NO value transposes needed
    - gate applied during PSUM eviction (DVE tensor_tensor vs broadcast gates)
    - dilated causal conv = 4 full-width (1024) DVE shifted fused ops, bf16
    - output written as [D, m] bf16; host re-transposes/upcasts
"""
import sys

sys.path.insert(0, "/opt/trn_rl_repo")

import numpy as np
import ml_dtypes

import concourse.bass as bass
import concourse.tile as tile
from concourse import mybir
from concourse.masks import make_identity
from concourse.bass_utils import run_bass_kernel_spmd

# problem shapes (hardcoded per spec)
L, B, D = 4096, 2, 2048
H, Dh = 12, 128
E = H * Dh  # 1536
N = 100000
K, DIL = 4, 4
EPS = 1e-6

NCORES = 8
LC = L // NCORES          # 512 l-positions per core
HALO = (K - 1) * DIL      # 12
LE = LC + HALO            # 524
M = LE * B                # 1048 valid tokens (l-major, b inner)
MP = 1152                 # padded to 9*128
MT = MP // 128            # 9 m-tiles
DT = D // 128             # 16 d-tiles
ET = E // 128             # 12 e-tiles
MOUT = LC * B             # 1024 output tokens per core
OFF = HALO * B            # 24 = first valid output token
# phase-2 m chunks (PSUM bank is 512 fp32); last chunk covers only the 24
# valid tokens past 1024 — columns [1048, 1152) are pad no consumer reads
CHUNKS = [(0, 512), (512, 512), (1024, 24)]

BF16 = mybir.dt.bfloat16
F32 = mybir.dt.float32
I32 = mybir.dt.int32

# scal columns per d-tile
SC_W0, SC_W1, SC_W2, SC_W3P, SC_CB = range(5)
NSC = 5


def _split_multi_waits(nc):
    """This walrus build accepts only one sync-wait per instruction; hoist
    extra waits onto injected NOPs on the same engine (order-preserving)."""
    for f in nc.m.functions:
        for bb in f.blocks:
            new_insts = []
            for inst in bb.instructions:
                si = inst.sync_info
                if si is not None and si.on_wait and len(si.on_wait) > 1:
                    for w in si.on_wait[:-1]:
                        nop = mybir.InstNoOp(
                            name=nc.get_next_instruction_name(), ins=[], outs=[]
                        )
                        nop.engine = inst.engine
                        nop.sync_info = mybir.SyncInfo(on_wait=[w], on_update=[])
                        new_insts.append(nop)
                    si.on_wait = [si.on_wait[-1]]
                new_insts.append(inst)
            bb.instructions = new_insts


def build_program(split_waits=True, debug_dumps=False):
    nc = bass.Bass("TRN2", target_bir_lowering=False, debug=False)
    demb = dgb = None
    if debug_dumps:
        demb = nc.declare_dram_parameter(
            "demb", [128, ET * MP], BF16, isOutput=True
        )
        dgb = nc.declare_dram_parameter("dgb", [128, MP], BF16, isOutput=True)

    tabs = nc.declare_dram_parameter(
        "tabs", [H * N + 16, Dh], BF16, isOutput=False
    )
    ids = nc.declare_dram_parameter("ids", [128, MT * H], I32, isOutput=False)
    hid = nc.declare_dram_parameter("hid", [MP, D], BF16, isOutput=False)
    wk = nc.declare_dram_parameter("wk", [E, D], BF16, isOutput=False)
    wv = nc.declare_dram_parameter("wv", [E, D], BF16, isOutput=False)
    scal = nc.declare_dram_parameter("scal", [128, DT * NSC], F32, isOutput=False)
    e9 = nc.declare_dram_parameter("e9", [128, MP], BF16, isOutput=False)
    outT = nc.declare_dram_parameter("outT", [D, MOUT], BF16, isOutput=True)

    AR = mybir.ActivationFunctionType
    ALU = mybir.AluOpType

    with tile.TileContext(nc) as tc:
        with (
            tc.tile_pool(name="persist", bufs=1) as pp,
            tc.tile_pool(name="work", bufs=3) as wp,
            tc.tile_pool(name="stat", bufs=2) as sp,
        ):
            # ---- constants / small inputs ----
            eps_sb = pp.tile([128, 1], F32, tag="eps")
            nc.vector.memset(eps_sb[:], EPS)

            ids_sb = pp.tile([128, MT * H], I32, tag="ids")
            nc.sync.dma_start(ids_sb[:], ids.ap())
            scal_sb = pp.tile([128, DT * NSC], F32, tag="scal")
            nc.sync.dma_start(scal_sb[:], scal.ap())
            e9_sb = pp.tile([128, MP], BF16, tag="e9")
            nc.sync.dma_start(e9_sb[:], e9.ap())

            def sc(dt_, c):
                return scal_sb[:, dt_ * NSC + c : dt_ * NSC + c + 1]

            # ---- weights: wk first (phase 1 needs it), wv during phase 1 ----
            wk_sb = pp.tile([128, ET, D], BF16, tag="wk")
            for e in range(ET):
                nc.scalar.dma_start(wk_sb[:, e, :], wk[e * 128 : (e + 1) * 128, :])
            wv_sb = pp.tile([128, ET, D], BF16, tag="wv")
            for e in range(ET):
                nc.scalar.dma_start(wv_sb[:, e, :], wv[e * 128 : (e + 1) * 128, :])

            # ---- gathers: ONE batched indirect DMA per m-tile ----
            bc_reg = nc.gpsimd.to_reg(H * N - 1)
            emb_raws = []
            for t in range(MT):
                er = wp.tile(
                    [128, H, Dh], BF16, tag="emb_raw", bufs=6,
                    name=f"emb_raw{t}",
                )
                if t in (0, MT - 1):
                    nc.gpsimd.memset(er[:, :, :], 0)
                # one instruction per (tile, head): multi-column offset APs
                # mis-bind on this walrus build (all collapse onto the last
                # instruction's ids), so stay on the single-column path
                for h in range(H):
                    nc.gpsimd.indirect_dma_start(
                        out=er[:, h, :],
                        out_offset=None,
                        in_=tabs[:],
                        in_offset=bass.IndirectOffsetOnAxis(
                            ap=ids_sb[:, t * H + h : t * H + h + 1], axis=0
                        ),
                        bounds_check=bc_reg,
                        oob_is_err=False,
                    )
                emb_raws.append(er)

            ident = pp.tile([128, 128], BF16, tag="ident")
            make_identity(nc, ident[:])
            emb_sb = pp.tile([128, ET, MP], BF16, tag="emb")
            # per-tile stat columns (f32), consumed by the batched gate tail
            sk_a = pp.tile([128, MT], F32, tag="sk_a")
            sk_b = pp.tile([128, MT], F32, tag="sk_b")
            sh_t = pp.tile([128, MT], F32, tag="sh_t")
            pk_a = pp.tile([128, MT], F32, tag="pk_a")
            pk_b = pp.tile([128, MT], F32, tag="pk_b")

            # ================= PHASE 1 =================
            with tc.tile_pool(name="psum1", bufs=1, space="PSUM") as ps1:
                for t in range(MT):
                    er = emb_raws[t]
                    # transpose 12 head blocks -> emb_sb, 4 per PSUM eviction
                    for grp in range(3):
                        pt = ps1.tile(
                            [128, 512], BF16, tag="tpose", bufs=2, space="PSUM"
                        )
                        for j in range(4):
                            h = grp * 4 + j
                            nc.tensor.transpose(
                                out=pt[:, j * 128 : (j + 1) * 128],
                                in_=er[:, h, :],
                                identity=ident[:],
                            )
                        nc.scalar.copy(
                            out=emb_sb[
                                :, grp * 4 : (grp + 1) * 4, t * 128 : (t + 1) * 128
                            ],
                            in_=pt[:],
                        )

                    # hidden rows for this m-tile + h^2 accum
                    h_md = wp.tile([128, D], BF16, tag="h_md", bufs=2)
                    nc.sync.dma_start(h_md[:], hid.ap()[t * 128 : (t + 1) * 128, :])
                    hsj = wp.tile([128, D], BF16, tag="junk", bufs=2, name="hsj")
                    nc.scalar.activation(
                        out=hsj[:], in_=h_md[:], func=AR.Square,
                        accum_out=sh_t[:, t : t + 1],
                    )

                    # k matmul in 2 col-groups of 1024 (2 PSUM banks each)
                    for g in range(2):
                        kp = ps1.tile(
                            [128, 1024], F32, tag="kp", bufs=3, space="PSUM"
                        )
                        for e in range(ET):
                            for b in range(2):
                                nc.tensor.matmul(
                                    out=kp[:, b * 512 : (b + 1) * 512],
                                    lhsT=emb_sb[:, e, t * 128 : (t + 1) * 128],
                                    rhs=wk_sb[
                                        :, e,
                                        g * 1024 + b * 512 : g * 1024 + (b + 1) * 512,
                                    ],
                                    start=(e == 0), stop=(e == ET - 1),
                                )
                        # k stats: sum k^2 (ACT), sum k*h (DVE)
                        ksj = wp.tile(
                            [128, 1024], BF16, tag="junk", bufs=2, name="ksj"
                        )
                        sk_dst = sk_a if g == 0 else sk_b
                        nc.scalar.activation(
                            out=ksj[:], in_=kp[:], func=AR.Square,
                            accum_out=sk_dst[:, t : t + 1],
                        )
                        khj = wp.tile(
                            [128, 1024], BF16, tag="junk", bufs=2, name="khj"
                        )
                        pk_dst = pk_a if g == 0 else pk_b
                        nc.vector.scalar_tensor_tensor(
                            out=khj[:], in0=kp[:], scalar=1.0,
                            in1=h_md[:, g * 1024 : (g + 1) * 1024],
                            op0=ALU.mult, op1=ALU.mult,
                            accum_out=pk_dst[:, t : t + 1],
                        )

            # ============ GATE TAIL (batched over 9 tiles) ============
            sk = sp.tile([128, MT], F32, tag="sk")
            nc.vector.tensor_add(sk[:], sk_a[:], sk_b[:])
            s1 = sp.tile([128, MT], F32, tag="s1")
            nc.scalar.activation(
                out=s1[:], in_=sk[:], func=AR.Identity,
                bias=eps_sb[:, 0:1], scale=1.0 / D,
            )
            s2 = sp.tile([128, MT], F32, tag="s2")
            nc.scalar.activation(
                out=s2[:], in_=sh_t[:], func=AR.Identity,
                bias=eps_sb[:, 0:1], scale=1.0 / D,
            )
            tt = sp.tile([128, MT], F32, tag="tt")
            nc.vector.tensor_mul(tt[:], s1[:], s2[:])
            rr = sp.tile([128, MT], F32, tag="rr")
            nc.vector.reciprocal(rr[:], tt[:])
            rq = sp.tile([128, MT], F32, tag="rq")
            nc.scalar.activation(out=rq[:], in_=rr[:], func=AR.Sqrt)
            pks = sp.tile([128, MT], F32, tag="pks")
            nc.vector.tensor_add(pks[:], pk_a[:], pk_b[:])
            uu = sp.tile([128, MT], F32, tag="uu")
            nc.vector.scalar_tensor_tensor(
                out=uu[:], in0=pks[:], scalar=float(1.0 / np.sqrt(D)),
                in1=rq[:], op0=ALU.mult, op1=ALU.mult,
            )
            ab = sp.tile([128, MT], F32, tag="ab")
            nc.scalar.activation(out=ab[:], in_=uu[:], func=AR.Abs)
            mx = sp.tile([128, MT], F32, tag="mx")
            nc.vector.tensor_scalar_max(out=mx[:], in0=ab[:], scalar1=1e-6)
            r2 = sp.tile([128, MT], F32, tag="r2")
            nc.vector.reciprocal(r2[:], mx[:])
            q2 = sp.tile([128, MT], F32, tag="q2")
            nc.scalar.activation(out=q2[:], in_=r2[:], func=AR.Sqrt)
            st = sp.tile([128, MT], F32, tag="st")
            nc.vector.tensor_mul(st[:], uu[:], q2[:])
            g_t = sp.tile([128, MT], BF16, tag="g_t")
            nc.scalar.activation(out=g_t[:], in_=st[:], func=AR.Sigmoid)

            # ================= PHASE 2 =================
            # the gate broadcast's PE ops (transpose + ones-block matmuls)
            # wait on the DVE/ACT gate tail; emit the first two d-tiles'
            # matmuls ahead of them so the PE queue never stalls there
            gb = pp.tile([128, MP], BF16, tag="gb")
            GB_CHUNKS = [(0, 512), (512, 512), (1024, 128)]
            with tc.tile_pool(name="psum2", bufs=1, space="PSUM") as ps2:

                def v_mms(dt_):
                    vps = []
                    for ci, (c0, cw) in enumerate(CHUNKS):
                        vp = ps2.tile(
                            [128, cw], F32, tag=f"vp{ci}", bufs=2, space="PSUM",
                            name=f"vp{ci}_{dt_}",
                        )
                        vps.append(vp)
                    for e in range(ET):
                        for ci, (c0, cw) in enumerate(CHUNKS):
                            nc.tensor.matmul(
                                out=vps[ci][:, 0:cw],
                                lhsT=wv_sb[:, e, dt_ * 128 : (dt_ + 1) * 128],
                                rhs=emb_sb[:, e, c0 : c0 + cw],
                                start=(e == 0), stop=(e == ET - 1),
                            )
                    return vps

                def v_tail(dt_, vps):
                    # gate-apply during PSUM eviction
                    vg = wp.tile([128, MP], BF16, tag="vg", bufs=3)
                    for ci, (c0, cw) in enumerate(CHUNKS):
                        nc.vector.tensor_mul(
                            vg[:, c0 : c0 + cw], vps[ci][:, 0:cw],
                            gb[:, c0 : c0 + cw],
                        )
                    # dilated causal depthwise conv: 4 full-width fused taps
                    a1 = wp.tile([128, MOUT], BF16, tag="a1", bufs=2)
                    nc.vector.tensor_scalar(
                        out=a1[:], in0=vg[:, 0:MOUT],
                        scalar1=sc(dt_, SC_W0), scalar2=sc(dt_, SC_CB),
                        op0=ALU.mult, op1=ALU.add,
                    )
                    a2 = wp.tile([128, MOUT], BF16, tag="a2", bufs=2)
                    nc.vector.scalar_tensor_tensor(
                        out=a2[:], in0=vg[:, 8 : 8 + MOUT],
                        scalar=sc(dt_, SC_W1), in1=a1[:],
                        op0=ALU.mult, op1=ALU.add,
                    )
                    a3 = wp.tile([128, MOUT], BF16, tag="a3", bufs=2)
                    nc.vector.scalar_tensor_tensor(
                        out=a3[:], in0=vg[:, 16 : 16 + MOUT],
                        scalar=sc(dt_, SC_W2), in1=a2[:],
                        op0=ALU.mult, op1=ALU.add,
                    )
                    ot = wp.tile([128, MOUT], BF16, tag="ot", bufs=2)
                    nc.vector.scalar_tensor_tensor(
                        out=ot[:], in0=vg[:, OFF : OFF + MOUT],
                        scalar=sc(dt_, SC_W3P), in1=a3[:],
                        op0=ALU.mult, op1=ALU.add,
                    )
                    nc.sync.dma_start(
                        outT[dt_ * 128 : (dt_ + 1) * 128, :], ot[:]
                    )

                held = [(dt_, v_mms(dt_)) for dt_ in range(2)]

                # gate broadcast: transpose gates, then ones-block matmuls
                gt_ps = ps2.tile([128, 128], BF16, tag="gt_ps", space="PSUM")
                nc.tensor.transpose(
                    out=gt_ps[0:MT, :], in_=g_t[:, 0:MT], identity=ident[:]
                )
                g_row = sp.tile([128, 128], BF16, tag="g_row")
                nc.scalar.copy(out=g_row[0:MT, :], in_=gt_ps[0:MT, :])
                for ci, (c0, cw) in enumerate(GB_CHUNKS):
                    gb_ps = ps2.tile(
                        [128, 512], F32, tag="gb_ps", bufs=1, space="PSUM"
                    )
                    for j in range(cw // 128):
                        t = c0 // 128 + j
                        nc.tensor.matmul(
                            out=gb_ps[:, j * 128 : (j + 1) * 128],
                            lhsT=e9_sb[0:MT, t * 128 : (t + 1) * 128],
                            rhs=g_row[0:MT, :],
                            start=True, stop=True,
                        )
                    nc.scalar.copy(out=gb[:, c0 : c0 + cw], in_=gb_ps[:, 0:cw])

                if debug_dumps:
                    nc.sync.dma_start(demb.ap(), emb_sb[:, :, :])
                    nc.sync.dma_start(dgb.ap(), gb[:])

                for dt_, vps in held:
                    v_tail(dt_, vps)
                for dt_ in range(2, DT):
                    v_tail(dt_, v_mms(dt_))

    if split_waits:
        _split_multi_waits(nc)
    return nc


_CACHE = {}


def _get_program():
    if "nc" not in _CACHE:
        _CACHE["nc"] = build_program()
    return _CACHE["nc"]


def host_prep(hidden_states, hash_input_ids, emb_tables, key_w, key_b,
              norm1_w, norm2_w, value_w, value_b, conv_w, conv_b):
    """Shard + lay out inputs for the 8 cores. Returns in_maps list."""
    bf = ml_dtypes.bfloat16
    w12 = norm1_w.astype(np.float64) * norm2_w.astype(np.float64)
    assert np.allclose(w12, 1.0, atol=1e-5), (
        "fast path assumes norm1_w*norm2_w == 1 (problem spec: fill=ones)"
    )
    assert not key_b.any() and not value_b.any(), (
        "fast path assumes zero key/value biases (problem spec: fill=zeros)"
    )

    tabs_np = np.zeros((H * N + 16, Dh), bf)
    tabs_np[: H * N] = emb_tables.reshape(H * N, Dh).astype(bf)
    wk_np = np.ascontiguousarray(key_w.T).astype(bf)
    wv_np = np.ascontiguousarray(value_w.T).astype(bf)
    scal_d = np.empty((D, NSC), np.float32)
    scal_d[:, SC_W0] = conv_w[:, 0]
    scal_d[:, SC_W1] = conv_w[:, 1]
    scal_d[:, SC_W2] = conv_w[:, 2]
    scal_d[:, SC_W3P] = conv_w[:, 3] + 1.0
    scal_d[:, SC_CB] = conv_b
    scal_np = np.ascontiguousarray(
        scal_d.reshape(DT, 128, NSC).transpose(1, 0, 2).reshape(128, DT * NSC)
    )
    e9_np = np.zeros((128, MP), bf)
    for t in range(MT):
        e9_np[t, t * 128 : (t + 1) * 128] = 1.0

    head_off = (np.arange(H, dtype=np.int64) * N)[None, :]
    OOB = np.int32(H * N)

    in_maps = []
    for c in range(NCORES):
        l0 = c * LC
        lo = l0 - HALO
        lo_clip = max(lo, 0)
        nvalid = (l0 + LC) - lo_clip
        r0 = (lo_clip - lo) * B
        ids_c = np.full((MP, H), OOB, np.int32)
        seg = hash_input_ids[lo_clip : l0 + LC].reshape(nvalid * B, H)
        ids_c[r0 : r0 + nvalid * B] = (seg.astype(np.int64) + head_off).astype(
            np.int32
        )
        hid_c = np.zeros((MP, D), bf)
        hseg = hidden_states[lo_clip : l0 + LC].reshape(nvalid * B, D)
        hid_c[r0 : r0 + nvalid * B] = hseg.astype(bf)
        ids_r = np.ascontiguousarray(
            ids_c.reshape(MT, 128, H).transpose(1, 0, 2).reshape(128, MT * H)
        )
        in_maps.append(
            {
                "tabs": tabs_np,
                "ids": ids_r,
                "hid": hid_c,
                "wk": wk_np,
                "wv": wv_np,
                "scal": scal_np,
                "e9": e9_np,
            }
        )
    return in_maps


def unshard_output(results):
    """results: list of per-core dicts with 'outT' [D, MOUT] -> [L, B, D]."""
    out = np.empty((L, B, D), np.float32)
    for c in range(NCORES):
        o = results[c]["outT"].astype(np.float32)
        out[c * LC : (c + 1) * LC] = o.reshape(D, LC, B).transpose(1, 2, 0)
    return out


def kernel(hidden_states, hash_input_ids, emb_tables, key_w, key_b,
           norm1_w, norm2_w, value_w, value_b, conv_w, conv_b):
    args = [hidden_states, hash_input_ids, emb_tables, key_w, key_b,
            norm1_w, norm2_w, value_w, value_b, conv_w, conv_b]
    args = [np.asarray(a) for a in args]
    in_maps = host_prep(*args)
    nc = _get_program()
    res = run_bass_kernel_spmd(nc, in_maps, list(range(NCORES)))
    return unshard_output(res.results)
